# revision 41
# baseline (speedup 1.0000x reference)
"""Trainium2 Bass kernel for nn_Model_15418932592810 (Autoformer-style decoder layer).

Sharding: data-parallel over batch B=8 across the 8 NeuronCores (one batch
per core, no collectives). Within a core, activations are kept feature-major
(D on partitions) through the matmul chains; attention is computed keys-major
(scores^T) so the softmax denominators ride along as a ones-column in V; the
series-decomp / layernorm tail runs token-major.
"""
import math
import os
import numpy as np
import ml_dtypes

import concourse.bass as bass
import concourse.mybir as mybir
import concourse.tile as tile
from concourse import bacc
from concourse.bass_utils import run_bass_kernel_spmd

F32 = mybir.dt.float32
F32R = mybir.dt.float32r
BF16 = mybir.dt.bfloat16
AX = mybir.AxisListType
ALU = mybir.AluOpType
ACTF = mybir.ActivationFunctionType

B, L, D, H, DH, DFF, KMA = 8, 512, 512, 8, 64, 2048, 25
NT = 4          # number of 128-row tiles in a 512 dim
NF = DFF // 128
EPS = 1e-5
BF = ml_dtypes.bfloat16


def r(x):
    return x.bitcast(F32R)


# ----------------------------------------------------------------------------
# host-side input preparation
# ----------------------------------------------------------------------------

def mktile(pool, shape, dtype, tag, bufs=None):
    return pool.tile(shape, dtype, name=tag, tag=tag, bufs=bufs)


def _softplus(x):
    return np.logaddexp(0.0, x.astype(np.float64))


def _ma_matrix():
    pad = (KMA - 1) // 2
    A = np.zeros((L, L), dtype=np.float64)
    for i in range(L):
        for m in range(i, i + KMA):
            j = min(max(m - pad, 0), L - 1)
            A[i, j] += 1.0 / KMA
    return A.astype(np.float32)


def _row(x):
    return np.ascontiguousarray(np.asarray(x, dtype=np.float32).reshape(1, -1))


def _cols(x):
    n = np.asarray(x).shape[0]
    return np.ascontiguousarray(np.asarray(x, np.float32).reshape(n // 128, 128).T)


def _T(w):
    return np.ascontiguousarray(np.asarray(w, dtype=np.float32).T)


def _pack(a):
    # (R, N) with R=128*c -> (128, c*N): column block c holds rows [128c,128c+128)
    a = np.asarray(a)
    rr, n = a.shape
    c = rr // 128
    return np.ascontiguousarray(
        a.reshape(c, 128, n).transpose(1, 0, 2).reshape(128, c * n))


def _Tp(w):
    return _pack(_T(w))


def host_prepare(inputs):
    ins = {k: np.asarray(v) for k, v in inputs.items()}
    sh = {}
    s = 1.0 / math.sqrt(DH)

    qkv_w = ins["sa_qkv_w"].astype(np.float64)
    qkv_b = ins["sa_qkv_b"].astype(np.float64)
    attn_sets = [
        ("sa", qkv_w[:D] * s, qkv_b[:D] * s, qkv_w[D:2 * D], qkv_b[D:2 * D],
         qkv_w[2 * D:], qkv_b[2 * D:], ins["sa_out_w"], ins["sa_out_b"]),
        ("cf", ins["cf_q_w"] * s, ins["cf_q_b"] * s, ins["cf_k_w"], ins["cf_k_b"],
         ins["cf_v_w"], ins["cf_v_b"], ins["cf_o_w"], ins["cf_o_b"]),
        ("cr", ins["cr_q_w"] * s, ins["cr_q_b"] * s, ins["cr_k_w"], ins["cr_k_b"],
         ins["cr_v_w"], ins["cr_v_b"], ins["cr_o_w"], ins["cr_o_b"]),
    ]
    # K bias is softmax-invariant (adds a per-query constant to every logit);
    # V bias is folded through the out projection (sum of attn weights == 1):
    # bo' = bo + Wo @ bv.
    for p, wq, bq, wk, bk, wv, bv, wo, bo in attn_sets:
        sh[f"{p}_wqT"] = _Tp(wq).astype(BF)
        sh[f"{p}_bq_c"] = _cols(bq)
        sh[f"{p}_wkT"] = _Tp(wk).astype(BF)
        sh[f"{p}_wvT"] = _Tp(wv).astype(BF)
        sh[f"{p}_woT"] = _Tp(wo).astype(BF)
        bo_eff = (np.asarray(bo, np.float64)
                  + np.asarray(wo, np.float64) @ np.asarray(bv, np.float64))
        sh[f"{p}_bo_c"] = _cols(bo_eff)

    wq, wk, wv = np.split(ins["mha_in_w"], 3, axis=0)
    bq, bk, bv = np.split(ins["mha_in_b"], 3, axis=0)
    sh["mha_wkT"] = _Tp(wk).astype(BF)
    sh["mha_wvT"] = _Tp(wv).astype(BF)
    qvec = ((ins["global_q"].reshape(D) @ wq.astype(np.float64).T + bq) * s
            ).astype(np.float32)
    # per-tile lhsT for the single-query scores: column h of block c holds
    # q chunk c masked to head h's 64-row half (heads 2c, 2c+1 live in tile c)
    qv2 = np.zeros((128, 4, 8), np.float32)
    for c in range(NT):
        qv2[0:64, c, 2 * c] = qvec[128 * c:128 * c + 64]
        qv2[64:128, c, 2 * c + 1] = qvec[128 * c + 64:128 * (c + 1)]
    sh["mha_qv2"] = np.ascontiguousarray(qv2.reshape(128, 32)).astype(BF)
    sh["mha_woT"] = _Tp(ins["mha_out_w"]).astype(BF)
    mha_bo_eff = (np.asarray(ins["mha_out_b"], np.float64)
                  + np.asarray(ins["mha_out_w"], np.float64)
                  @ np.asarray(bv, np.float64))
    sh["mha_bo_row"] = _row(mha_bo_eff)

    sh["fus_w1T"] = _Tp(ins["fusion_w"][:, :D]).astype(BF)
    sh["fus_w2T"] = _Tp(ins["fusion_w"][:, D:]).astype(BF)
    sh["fus_b_row"] = _row(ins["fusion_b"])

    # norm3 gamma folds into conv1 columns; normf/norm3 beta cancels in my_ln
    sh["conv1T"] = _Tp(ins["conv1_w"] * ins["norm3_g"][None, :]).astype(BF)
    sh["conv2T"] = _Tp(ins["conv2_w"]).astype(BF)
    sh["trend_wT"] = _Tp(ins["trend_w"])
    sh["trend_b_row"] = _row(ins["trend_b"])
    sh["gf_rep"] = np.ascontiguousarray(
        np.tile(np.asarray(ins["normf_g"], np.float32)[None, :], (128, 1)))

    i = np.arange(L, dtype=np.float64)
    rel = i[None, :] - i[:, None]                  # rel[q, k] = k - q
    lf = _softplus(ins["sa_lam_f"])[:, None, None]
    lb = _softplus(ins["sa_lam_b"])[:, None, None]
    decay = np.where(rel[None] < 0, -lb * np.abs(rel[None]),
                     np.where(rel[None] > 0, -lf * rel[None], 0.0))
    # stored as exp(decay): the bias is applied multiplicatively after the
    # exp (exp(l+b) == exp(l)*exp(b)), saving a PE matmul per score tile
    sh["sa_biasT"] = np.stack(
        [_pack(m) for m in np.exp(decay).transpose(0, 2, 1)]).astype(BF)

    # cos(2*pi*w*(q-k)) = cos(wq)cos(wk) + sin(wq)sin(wk): rank-2 per head.
    for p, lw in [("cf", "cf_logw"), ("cr", "cr_logw")]:
        w = np.exp(ins[lw].astype(np.float64))[:, None]
        ang = 2.0 * math.pi * w * i[None, :]       # (H, L)
        cs = np.stack([np.cos(ang), np.sin(ang)], axis=1)  # (H, 2, L)
        sh[f"{p}_cs"] = np.ascontiguousarray(
            cs.transpose(1, 0, 2).reshape(2, H * L)).astype(BF)

    sh["A_lhsT"] = _Tp(_ma_matrix())                # lhsT[j, i] = A[i, j]
    sh["identf"] = np.eye(128, dtype=np.float32)
    sh["identb"] = np.eye(128, dtype=BF)
    sh["ones_row128"] = np.ones((1, 128), np.float32)
    sh["ones_col128"] = np.ones((128, 1), np.float32)
    sh["eps_col128"] = np.full((128, 1), EPS, np.float32)

    per_core = []
    for b in range(B):
        per_core.append({
            "xsa_tok": _pack(ins["x_sa"][b].astype(np.float32)),
            "xsaT": _Tp(ins["x_sa"][b]).astype(BF),
            "xq1T": _Tp(ins["x_q1"][b]).astype(BF),
            "xq2T": _Tp(ins["x_q2"][b]).astype(BF),
        })
    return sh, per_core


# ----------------------------------------------------------------------------
# program builder
# ----------------------------------------------------------------------------

def _attn_specs(p):
    sp = [
        (f"{p}_wqT", (128, 2048), BF16), (f"{p}_bq_c", (128, 4), F32),
        (f"{p}_wkT", (128, 2048), BF16),
        (f"{p}_wvT", (128, 2048), BF16),
        (f"{p}_woT", (128, 2048), BF16), (f"{p}_bo_c", (128, 4), F32),
    ]
    if p == "sa":
        sp.append((f"{p}_biasT", (8, 128, 2048), BF16))
    else:
        sp.append((f"{p}_cs", (2, 4096), BF16))
    return sp


SHARED_SPECS = (
    _attn_specs("sa") + _attn_specs("cf") + _attn_specs("cr") + [
        ("mha_wkT", (128, 2048), BF16),
        ("mha_wvT", (128, 2048), BF16),
        ("mha_qv2", (128, 32), BF16),
        ("mha_woT", (128, 2048), BF16), ("mha_bo_row", (1, 512), F32),
        ("fus_w1T", (128, 2048), BF16), ("fus_w2T", (128, 2048), BF16),
        ("fus_b_row", (1, 512), F32),
        ("conv1T", (128, 8192), BF16), ("conv2T", (128, 8192), BF16),
        ("trend_wT", (128, 2048), F32), ("trend_b_row", (1, 512), F32),
        ("gf_rep", (128, 512), F32),
        ("A_lhsT", (128, 2048), F32),
        ("identf", (128, 128), F32), ("identb", (128, 128), BF16),
        ("ones_row128", (1, 128), F32), ("ones_col128", (128, 1), F32),
        ("eps_col128", (128, 1), F32),
    ]
)
PER_CORE_SPECS = [
    ("xsa_tok", (128, 2048), F32), ("xsaT", (128, 2048), BF16),
    ("xq1T", (128, 2048), BF16), ("xq2T", (128, 2048), BF16),
]
OUT_SPECS = [("out_x", (512, 512), F32), ("out_trend", (512, 512), F32)]
DTYPES = {name: dt for name, _, dt in SHARED_SPECS + PER_CORE_SPECS}


def hslice(tiles, h):
    t = tiles[h // 2]
    off = 64 * (h % 2)
    return t[off:off + 64, :]


class Prog:
    def __init__(self, nc, tc, Hd):
        self.nc, self.tc, self.Hd = nc, tc, Hd

    def load_tiles(self, pool, name, n=NT, width=512, dtype=None, tag=None,
                   bufs=None, split=False):
        if dtype is None:
            dtype = DTYPES.get(name, F32)
        """One packed DMA (or n chunked DMAs when split=True, so the first
        consumer can start before the whole tensor lands); returns the n
        (128, width) chunk views."""
        nc = self.nc
        dram = self.Hd[name]
        t = mktile(pool, [128, n * width], dtype, tag or name, bufs=bufs)
        if split:
            for c in range(n):
                sl = slice(width * c, width * (c + 1))
                if dtype == F32:
                    nc.sync.dma_start(r(t[:, sl]), r(dram[:, sl]))
                else:
                    nc.sync.dma_start(t[:, sl], dram[:, sl])
        elif dtype == F32:
            nc.sync.dma_start(r(t[:]), r(dram[:]))
        else:
            nc.sync.dma_start(t[:], dram[:])
        return [t[:, width * c:width * (c + 1)] for c in range(n)]

    def rowload(self, pool, name, width=512, tag=None, bufs=None):
        t = mktile(pool, [1, width], F32, tag or name, bufs=bufs)
        self.nc.sync.dma_start(r(t[:]), r(self.Hd[name][:]))
        return t

    def colload(self, pool, name, n=NT, tag=None, bufs=None):
        t = mktile(pool, [128, n], F32, tag or name, bufs=bufs)
        self.nc.sync.dma_start(r(t[:]), r(self.Hd[name][:]))
        return t

    # ------------------------------------------------------------------
    def proj_fm(self, psp, sbp, xT, wT_sb, b_c, tag):
        """Feature-major projection: out^T = W @ x^T + b, NT tiles (128,512).
        bf16 in, bf16 out."""
        nc = self.nc
        n_out = wT_sb[0].shape[1]
        outs = []
        for m in range(n_out // 128):
            ps = mktile(psp, [128, 512], F32, "mm_ps")
            for kc in range(NT):
                nc.tensor.matmul(ps[:], wT_sb[kc][:, 128 * m:128 * (m + 1)],
                                 xT[kc][:], start=(kc == 0), stop=(kc == NT - 1))
            o = mktile(sbp, [128, 512], BF16, f"{tag}_{m}")
            if b_c is None:
                nc.any.tensor_copy(o[:], ps[:])
            else:
                nc.vector.tensor_scalar(o[:], ps[:], b_c[:, m:m + 1], None,
                                        op0=ALU.add)
            outs.append(o)
        return outs

    def proj_tok_aug(self, psp, sbp, xT, wvT_sb, tag):
        """Token-major V projection with a ones column interleaved per head.
        The V bias is folded into the out-projection bias host-side."""
        nc = self.nc
        outs = []
        for kc in range(NT):
            ps = mktile(psp, [128, 512], F32, "mm_ps")
            for fc in range(NT):
                nc.tensor.matmul(ps[:], xT[fc][:, 128 * kc:128 * (kc + 1)],
                                 wvT_sb[fc][:], start=(fc == 0),
                                 stop=(fc == NT - 1))
            o = mktile(sbp, [128, 520], BF16, f"{tag}_{kc}")
            nc.vector.memset(o[:], 1.0)
            ov = o[:].rearrange("p (h c) -> p h c", c=65)
            nc.vector.tensor_copy(
                ov[:, :, 0:64], ps[:].rearrange("p (h c) -> p h c", c=64))
            outs.append(o)
        return outs

    def attention(self, name, qin, kvin, resid_q=False, preq_w=None):
        """8-head attention with additive bias; returns feature-major NT tiles
        allocated in the persistent pool. qin/kvin: DRAM name or tile list."""
        nc, tc, Hd = self.nc, self.tc, self.Hd
        outs = [mktile(self.aopool, [128, 512], BF16, f"{name}_out_{m}")
                for m in range(NT)]
        with (
            tc.tile_pool(name=f"{name}_ps", bufs=2, space="PSUM") as psp,
            tc.tile_pool(name=f"{name}_sb", bufs=1) as sbp,
        ):
            wp = self.wpool
            if isinstance(qin, str):
                qinT = self.load_tiles(wp, qin, tag="at_inq", bufs=2)
            else:
                qinT = qin
            if preq_w is None:
                wqT = self.load_tiles(wp, f"{name}_wqT", tag="at_wq", bufs=2)
            else:
                wqT = preq_w
            residT = qinT if resid_q else None
            bq_c = self.colload(wp, f"{name}_bq_c", tag="at_bq", bufs=2)
            if isinstance(kvin, str):
                kvinT = (qinT if kvin == qin else
                         self.load_tiles(wp, kvin, tag="at_inkv", bufs=2))
            else:
                kvinT = kvin
            wkT = self.load_tiles(wp, f"{name}_wkT", tag="at_wk", bufs=2)
            wvT = self.load_tiles(wp, f"{name}_wvT", tag="at_wv", bufs=2)
            woT = self.load_tiles(wp, f"{name}_woT", tag="at_wo", bufs=1)
            bo_c = self.colload(wp, f"{name}_bo_c", tag="at_bo", bufs=2)
            if name == "sa":
                biasT = Hd[f"{name}_biasT"]
                cs = None
            else:
                biasT = None
                cs = mktile(sbp, [2, 4096], BF16, "at_cs")
                nc.sync.dma_start(cs[:], Hd[f"{name}_cs"][:])

            qT = self.proj_fm(psp, sbp, qinT, wqT, bq_c, "at_q")
            kT = self.proj_fm(psp, sbp, kvinT, wkT, None, "at_k")
            vaug = self.proj_tok_aug(psp, sbp, kvinT, wvT, "at_v")

            aout = []
            for half in range(2):
                expT = [mktile(sbp, [128, 2048], BF16, f"at_exp_{kc}", bufs=1)
                        for kc in range(NT)]
                for h4 in range(4):
                    h = 4 * half + h4
                    if biasT is not None:
                        bsl = mktile(sbp, [128, 2048], BF16, "at_bias", bufs=2)
                        nc.sync.dma_start(bsl[:], biasT[h])
                    for kc in range(NT):
                        ps = mktile(psp, [128, 512], F32, "sc_ps")
                        if biasT is None:
                            nc.tensor.matmul(
                                ps[:],
                                cs[:, 512 * h + 128 * kc:
                                     512 * h + 128 * (kc + 1)],
                                cs[:, 512 * h:512 * (h + 1)],
                                start=True, stop=False)
                        nc.tensor.matmul(
                            ps[:],
                            hslice(kT, h)[:, 128 * kc:128 * (kc + 1)],
                            hslice(qT, h), start=(biasT is not None),
                            stop=True)
                        esl = expT[kc][:, 512 * h4:512 * (h4 + 1)]
                        nc.scalar.activation(esl, ps[:], ACTF.Exp)
                        if biasT is not None:
                            nc.vector.tensor_tensor(
                                esl, esl, bsl[:, 512 * kc:512 * (kc + 1)],
                                op=ALU.mult)
                # AV with the softmax denominator riding as the 65th row
                # per head. Each h4 group gets its own PSUM bank so the next
                # group's matmuls don't serialize against this group's
                # reciprocal/broadcast/normalize readers.
                for h4 in range(4):
                    h = 4 * half + h4
                    av = mktile(psp, [65, 512], F32, "av", bufs=4)
                    for kc in range(NT):
                        nc.tensor.matmul(av[:],
                                         vaug[kc][:, 65 * h:65 * h + 65],
                                         expT[kc][:, 512 * h4:512 * (h4 + 1)],
                                         start=(kc == 0), stop=(kc == NT - 1))
                    recip = mktile(sbp, [1, 512], F32, "at_recip", bufs=2)
                    nc.vector.reciprocal(recip[:], av[64:65, :])
                    rep = mktile(sbp, [64, 512], F32, "at_rep", bufs=2)
                    nc.gpsimd.partition_broadcast(rep[:], recip[:])
                    if h % 2 == 0:
                        pair = mktile(sbp, [128, 512], BF16, f"at_ao_{h // 2}")
                        aout.append(pair)
                    off = 64 * (h % 2)
                    nc.vector.tensor_tensor(aout[h // 2][off:off + 64, :],
                                            av[0:64, :], rep[:], op=ALU.mult)

            for m in range(NT):
                ps = mktile(psp, [128, 512], F32, "mm_ps")
                for c in range(NT):
                    nc.tensor.matmul(ps[:],
                                     woT[c][:, 128 * m:128 * (m + 1)],
                                     aout[c][:], start=(c == 0),
                                     stop=(c == NT - 1))
                nc.vector.tensor_scalar(outs[m][:], ps[:],
                                        bo_c[:, m:m + 1], None, op0=ALU.add)
                if residT is not None:
                    nc.vector.tensor_tensor(outs[m][:], outs[m][:],
                                            residT[m][:], op=ALU.add)
        return outs

    # ------------------------------------------------------------------
    def ctx_pool_fusion(self, ca1outT, sa_outT, wkT, qv2, xsa_tok):
        """MultiheadAttention pooling with precomputed 1-token query.

        Single-query attention is computed head-major: scores land as an
        (8, 512) tile (one head per partition) via per-chunk masked-q lhsT,
        so exp/sum/normalize all run on the right axes with no broadcasts.
        The fusion W1 matmuls are emitted mid-chain to keep PE busy during
        the exp/transpose latency. Returns (s2b row, fusion tiles)."""
        nc, tc = self.nc, self.tc
        s2b = mktile(self.keep, [1, 512], F32, "s2b_row")
        f_tiles = [mktile(self.keep, [128, 512], F32, f"fus_{c}")
                   for c in range(NT)]
        with (
            tc.tile_pool(name="mha_ps", bufs=2, space="PSUM") as psp,
            tc.tile_pool(name="mha_sb", bufs=1) as sbp,
        ):
            wvT = self.load_tiles(sbp, "mha_wvT")
            w1T = self.load_tiles(sbp, "fus_w1T")
            woT = self.load_tiles(sbp, "mha_woT")
            bo_row = self.rowload(sbp, "mha_bo_row")
            w2T = self.load_tiles(sbp, "fus_w2T")
            fus_b_row = self.rowload(sbp, "fus_b_row")

            kT = self.proj_fm(psp, sbp, ca1outT, wkT, None, "mha_k")

            sc_ps = mktile(psp, [8, 512], F32, "sc_y_ps", bufs=1)
            for c in range(NT):
                nc.tensor.matmul(sc_ps[:], qv2[:, 8 * c:8 * (c + 1)],
                                 kT[c][:], start=(c == 0),
                                 stop=(c == NT - 1))
            e = mktile(sbp, [8, 512], F32, "mha_e")
            zsum = mktile(sbp, [8, 1], F32, "mha_z")
            nc.scalar.activation(e[:], sc_ps[:], ACTF.Exp, accum_out=zsum[:])
            rz = mktile(sbp, [8, 1], F32, "mha_rz")
            nc.vector.reciprocal(rz[:], zsum[:])

            # V projection fills PE while the exp/transpose chain drains
            v_tok = []
            for kc in range(NT):
                ps = mktile(psp, [128, 512], F32, "mm_ps")
                for fc in range(NT):
                    nc.tensor.matmul(ps[:],
                                     ca1outT[fc][:, 128 * kc:128 * (kc + 1)],
                                     wvT[fc][:], start=(fc == 0),
                                     stop=(fc == NT - 1))
                o = mktile(sbp, [128, 512], BF16, f"mha_v_{kc}")
                nc.vector.tensor_copy(o[:], ps[:])
                v_tok.append(o)

            t_ps = mktile(psp, [128, 32], F32, "mha_t_ps", bufs=1)
            for c in range(NT):
                nc.tensor.transpose(t_ps[:, 8 * c:8 * (c + 1)],
                                    e[0:8, 128 * c:128 * (c + 1)],
                                    self.identf[0:8, 0:8])
            attnT = mktile(sbp, [128, 32], BF16, "mha_attnT")
            nc.vector.tensor_copy(attnT[:], t_ps[:])
            y_ps = mktile(psp, [8, 512], F32, "sc_y_ps", bufs=1)
            for c in range(NT):
                nc.tensor.matmul(y_ps[:], attnT[:, 8 * c:8 * (c + 1)],
                                 v_tok[c][:], start=(c == 0),
                                 stop=(c == NT - 1))
            y_sb = mktile(sbp, [8, 512], F32, "mha_y")
            nc.vector.tensor_scalar(y_sb[:], y_ps[:], rz[:, 0:1], None,
                                    op0=ALU.mult)

            # gather y into feature-major (head h's 64-dim block of row h)
            t2 = mktile(psp, [128, 32], F32, "mha_t_ps", bufs=1)
            for c in range(NT):
                nc.tensor.transpose(t2[:, 8 * c:8 * (c + 1)],
                                    y_sb[0:8, 128 * c:128 * (c + 1)],
                                    self.identf[0:8, 0:8])
            y_fm = mktile(sbp, [128, NT], BF16, "mha_yfm")
            for c in range(NT):
                nc.vector.tensor_copy(
                    y_fm[0:64, c:c + 1],
                    t2[0:64, 8 * c + 2 * c:8 * c + 2 * c + 1])
                nc.vector.tensor_copy(
                    y_fm[64:128, c:c + 1],
                    t2[64:128, 8 * c + 2 * c + 1:8 * c + 2 * c + 2])

            s_ps = mktile(psp, [1, 512], F32, "mha_s_ps", bufs=1)
            nc.tensor.matmul(s_ps[:], r(self.ones1x1[:]), r(bo_row[:]),
                             start=True, stop=False)
            for c in range(NT):
                nc.tensor.matmul(s_ps[:], y_fm[:, c:c + 1], woT[c][:],
                                 start=False, stop=(c == NT - 1))
            s_row = mktile(sbp, [1, 512], F32, "mha_srow")
            nc.vector.tensor_copy(s_row[:], s_ps[:])
            t3 = mktile(psp, [128, 32], F32, "mha_t_ps", bufs=1)
            for c in range(NT):
                nc.tensor.transpose(t3[:, c:c + 1],
                                    s_row[0:1, 128 * c:128 * (c + 1)],
                                    self.identf[0:1, 0:1])
            s_fm = mktile(sbp, [128, NT], BF16, "mha_sfm")
            nc.vector.tensor_copy(s_fm[:], t3[:, 0:NT])
            s2_ps = mktile(psp, [1, 512], F32, "mha_s_ps", bufs=1)
            nc.tensor.matmul(s2_ps[:], r(self.ones1x1[:]), r(fus_b_row[:]),
                             start=True, stop=False)
            for c in range(NT):
                nc.tensor.matmul(s2_ps[:], s_fm[:, c:c + 1], w2T[c][:],
                                 start=False, stop=(c == NT - 1))
            nc.vector.tensor_copy(r(s2b[:]), s2_ps[:])

            # fusion W1 matmuls, with the x_sa residual (identity matmul)
            # and the ctx-pool row (K=1 ones broadcast) accumulated in PSUM:
            # the tiles coming out are the finished pre-normf activations
            for tc_ in range(NT):
                ps = mktile(psp, [128, 512], F32, "mm_ps")
                nc.tensor.matmul(ps[:], r(self.identf[:]),
                                 r(xsa_tok[tc_][:]), start=True, stop=False)
                nc.tensor.matmul(ps[:], r(self.ones1x128[:]), r(s2b[:]),
                                 start=False, stop=False)
                for fc in range(NT):
                    nc.tensor.matmul(
                        ps[:], sa_outT[fc][:, 128 * tc_:128 * (tc_ + 1)],
                        w1T[fc][:], start=False, stop=(fc == NT - 1))
                if tc_ % 2:
                    nc.any.tensor_copy(r(f_tiles[tc_][:]), ps[:])
                else:
                    nc.vector.tensor_copy(r(f_tiles[tc_][:]), ps[:])
        return s2b, f_tiles

    # ------------------------------------------------------------------
    def my_ln(self, psp, sbp, out_pool, y_tiles, tag, gmul=None, outs=None):
        """Full my_Layernorm: per-token stats + normalize, then subtract the
        per-sequence mean. Everything is per-tile pipelined: tile c flows
        stats -> normalize -> seq-sum matmul while tile c+1 computes stats,
        so the serial chain exposed to PE is ~one tile, not four."""
        nc = self.nc
        ps = mktile(psp, [1, 512], F32, "sm_ps", bufs=1)
        xh = []
        for c in range(NT):
            sums = mktile(sbp, [128, 1], F32, f"{tag}_sums", bufs=4)
            nc.vector.tensor_reduce(sums[:], y_tiles[c][:], axis=AX.X,
                                    op=ALU.add)
            sq = mktile(sbp, [128, 512], F32, "lnsq", bufs=2)
            sumsq = mktile(sbp, [128, 1], F32, f"{tag}_sumsq", bufs=4)
            nc.scalar.activation(sq[:], y_tiles[c][:], ACTF.Square,
                                 accum_out=sumsq[:])
            mu = mktile(sbp, [128, 1], F32, f"{tag}_mu", bufs=4)
            nc.vector.tensor_scalar(mu[:], sums[:], 1.0 / D, None,
                                    op0=ALU.mult)
            var = mktile(sbp, [128, 1], F32, f"{tag}_var", bufs=4)
            nc.vector.tensor_tensor(var[:], mu[:], mu[:], op=ALU.mult)
            msq = mktile(sbp, [128, 1], F32, f"{tag}_msq", bufs=4)
            nc.vector.tensor_scalar(msq[:], sumsq[:], 1.0 / D, None,
                                    op0=ALU.mult)
            nc.vector.tensor_tensor(var[:], msq[:], var[:], op=ALU.subtract)
            std = mktile(sbp, [128, 1], F32, f"{tag}_std", bufs=4)
            nc.scalar.activation(std[:], var[:], ACTF.Sqrt,
                                 bias=self.eps_col[:, 0:1])
            rstd = mktile(sbp, [128, 1], F32, f"{tag}_rstd", bufs=4)
            nc.vector.reciprocal(rstd[:], std[:])
            o = mktile(sbp, [128, 512], F32, "ln_xh", bufs=4)
            nc.vector.tensor_scalar(r(o[:]), y_tiles[c][:], mu[:],
                                    rstd[:], op0=ALU.subtract, op1=ALU.mult)
            nc.tensor.matmul(ps[:], r(self.ones_col[:, 0:1]), r(o[:]),
                             start=(c == 0), stop=(c == NT - 1))
            xh.append(o)
        row = mktile(sbp, [1, 512], F32, f"{tag}_row")
        nc.scalar.mul(r(row[:]), ps[:], 1.0 / L)
        rep_ps = mktile(psp, [128, 512], F32, "bcast_ps", bufs=1)
        nc.tensor.matmul(rep_ps[:], r(self.ones1x128[:]), r(row[:]),
                         start=True, stop=True)
        if outs is None:
            outs = [mktile(out_pool, [128, 512], F32, f"{tag}_out_{c}")
                    for c in range(NT)]
        for c in range(NT):
            nc.vector.tensor_tensor(r(outs[c][:]), xh[c][:], rep_ps[:],
                                    op=ALU.subtract)
            if gmul is not None:
                eng = nc.gpsimd if c % 2 else nc.vector
                eng.tensor_tensor(r(outs[c][:]), outs[c][:], gmul[:],
                                  op=ALU.mult)
        return outs

    def seqmean_sub(self, psp, sbp, out_pool, xh_tiles, tag, gmul=None,
                    outs=None):
        """Subtract the per-sequence (partition-axis) mean. The mean row is
        replicated to 128 partitions with a K=1 PE matmul into PSUM (~0.2us)
        instead of a gpsimd partition_broadcast (~3us)."""
        nc = self.nc
        ps = mktile(psp, [1, 512], F32, "sm_ps", bufs=1)
        for c in range(NT):
            nc.tensor.matmul(ps[:], r(self.ones_col[:, 0:1]), r(xh_tiles[c][:]),
                             start=(c == 0), stop=(c == NT - 1))
        row = mktile(sbp, [1, 512], F32, f"{tag}_row")
        nc.scalar.mul(row[:], ps[:], 1.0 / L)
        rep_ps = mktile(psp, [128, 512], F32, "bcast_ps", bufs=1)
        nc.tensor.matmul(rep_ps[:], r(self.ones1x128[:]), r(row[:]),
                         start=True, stop=True)
        if outs is None:
            outs = [mktile(out_pool, [128, 512], F32, f"{tag}_out_{c}")
                    for c in range(NT)]
        for c in range(NT):
            nc.vector.tensor_tensor(r(outs[c][:]), xh_tiles[c][:], rep_ps[:],
                                    op=ALU.subtract)
            if gmul is not None:
                nc.vector.tensor_tensor(r(outs[c][:]), outs[c][:], gmul[:],
                                        op=ALU.mult)
        return outs

    def transpose_512(self, psp, sbp, src_tiles, tag):
        nc = self.nc
        outs = [mktile(sbp, [128, 512], F32, f"{tag}_{c}") for c in range(NT)]
        for rr in range(NT):
            for cc in range(NT):
                ps = mktile(psp, [128, 128], F32, "tr_ps")
                nc.tensor.transpose(ps[:],
                                    src_tiles[rr][:, 128 * cc:128 * (cc + 1)],
                                    self.identf[:])
                nc.any.tensor_copy(r(outs[cc][:, 128 * rr:128 * (rr + 1)]),
                                   ps[:])
        return outs

    def ma_matmul(self, psp, A_lhsT, x_tiles):
        """A is banded (half-bandwidth 12), so tile block (tc, kc) is zero
        unless |tc - kc| <= 1: 10 matmuls instead of 16."""
        nc = self.nc
        pss = []
        for tc_ in range(NT):
            kcs = [kc for kc in range(NT) if abs(kc - tc_) <= 1]
            ps = mktile(psp, [128, 512], F32, "ma_ps", bufs=2)
            for i, kc in enumerate(kcs):
                nc.tensor.matmul(ps[:],
                                 r(A_lhsT[kc][:, 128 * tc_:128 * (tc_ + 1)]),
                                 r(x_tiles[kc][:]), start=(i == 0),
                                 stop=(i == len(kcs) - 1))
            pss.append(ps)
        return pss


def emit(tc, nc, Hd):
    p = Prog(nc, tc, Hd)
    with tc.tile_pool(name="keep", bufs=1) as keep:
        p.keep = keep
        # three attention blocks; their outputs live in a scoped pool that
        # closes before the tail pools open (frees 24KB/partition)
        ao_cm = tc.tile_pool(name="ao", bufs=1)
        p.aopool = ao_cm.__enter__()
        ctxw_cm = tc.tile_pool(name="ctxw", bufs=1)
        ctxw = ctxw_cm.__enter__()
        with tc.tile_pool(name="wts", bufs=2) as wpool:
            p.wpool = wpool
            # the first projection's operands head the DMA queue; constants
            # (needed only ~15us in) follow them
            sa_qt = mktile(wpool, [128, 2048], BF16, "at_inq", bufs=2)
            sa_wt = mktile(wpool, [128, 2048], BF16, "at_wq", bufs=2)
            for c in range(NT):
                sl = slice(512 * c, 512 * (c + 1))
                nc.sync.dma_start(sa_wt[:, sl], Hd["sa_wqT"][:, sl])
                nc.sync.dma_start(sa_qt[:, sl], Hd["xsaT"][:, sl])
            sa_qinT = [sa_qt[:, 512 * c:512 * (c + 1)] for c in range(NT)]
            sa_wqT = [sa_wt[:, 512 * c:512 * (c + 1)] for c in range(NT)]
            p.ones1x128 = mktile(keep, [1, 128], F32, "ones1x128")
            nc.sync.dma_start(r(p.ones1x128[:]), r(Hd["ones_row128"][:]))
            p.ones1x1 = p.ones1x128[0:1, 0:1]
            p.ones_col = mktile(keep, [128, 1], F32, "ones_col")
            nc.sync.dma_start(r(p.ones_col[:]), r(Hd["ones_col128"][:]))
            p.eps_col = mktile(keep, [128, 1], F32, "eps_col")
            nc.sync.dma_start(p.eps_col[:], Hd["eps_col128"][:])
            p.identf = mktile(keep, [128, 128], F32, "identf")
            nc.sync.dma_start(r(p.identf[:]), r(Hd["identf"][:]))
            sa_outT = p.attention("sa", sa_qinT, sa_qinT, preq_w=sa_wqT)
            xq1T = p.load_tiles(keep, "xq1T")
            xsa_tok = p.load_tiles(keep, "xsa_tok")
            enrichedT = p.attention("cf", "xq2T", xq1T, resid_q=True)
            # prefetch the ctx-pool K weights + query so kproj can start the
            # moment the cr out-projection lands
            mha_wkT = p.load_tiles(ctxw, "mha_wkT")
            mha_qv2 = mktile(ctxw, [128, 32], BF16, "mha_qv2")
            nc.sync.dma_start(mha_qv2[:], Hd["mha_qv2"][:])
            ca1outT = p.attention("cr", xq1T, enrichedT)

        # context pooling + fusion W1 matmuls (interleaved inside)
        s2b, f_tiles = p.ctx_pool_fusion(ca1outT, sa_outT,
                                         mha_wkT, mha_qv2, xsa_tok)
        ctxw_cm.__exit__(None, None, None)
        ao_cm.__exit__(None, None, None)

        # persistent tail tensors
        xln = [mktile(keep, [128, 512], F32, f"xln_{c}") for c in range(NT)]
        xs = [mktile(keep, [128, 512], F32, f"xs_{c}") for c in range(NT)]

        with (
            tc.tile_pool(name="tail_ps", bufs=2, space="PSUM") as psp,
            tc.tile_pool(name="tail_sb", bufs=1) as sbp,
        ):
            gf_rep = mktile(sbp, [128, 512], F32, "gf_rep")
            nc.sync.dma_start(gf_rep[:], Hd["gf_rep"][:])
            A_lhsT = p.load_tiles(sbp, "A_lhsT")
            # conv weights (bf16) land while the pre-FFN tail chain runs
            c1T = p.load_tiles(sbp, "conv1T", width=2048, dtype=BF16)
            c2T = p.load_tiles(sbp, "conv2T", n=NF, dtype=BF16)

            y_tiles = f_tiles

            # normf (token-major my_ln); beta cancels, gamma applied
            p.my_ln(psp, sbp, sbp, y_tiles, "lnf", gmul=gf_rep, outs=xln)

            # decomp1: xs = xln - A @ xln; keep the trend part for the
            # trend head (A@(xln+y2) == t1 + t2 by linearity)
            t1_ps = p.ma_matmul(psp, A_lhsT, xln)
            t1_sb = []
            for c in range(NT):
                nc.vector.tensor_tensor(xs[c][:], xln[c][:], t1_ps[c][:],
                                        op=ALU.subtract)
                t1c = mktile(sbp, [128, 512], F32, f"t1_{c}")
                nc.any.tensor_copy(t1c[:], t1_ps[c][:])
                t1_sb.append(t1c)

            # norm3 (gamma folded into conv1, beta cancels)
            xn = p.my_ln(psp, sbp, sbp, xs, "ln3")

            # ffn: transpose -> conv1+relu (feature-major) -> conv2 (token-major)
            # conv weights are bf16 and were prefetched in the fw pool.
            relu = []
            xnT = []
            for c in range(NT):
                t = mktile(sbp, [128, 512], BF16, f"xnT_{c}")
                xnT.append(t)
            for rr in range(NT):
                for cc in range(NT):
                    tp = mktile(psp, [128, 128], F32, "tr_ps")
                    nc.tensor.transpose(tp[:],
                                        xn[rr][:, 128 * cc:128 * (cc + 1)],
                                        p.identf[:])
                    nc.any.tensor_copy(xnT[cc][:, 128 * rr:128 * (rr + 1)],
                                       tp[:])
            for m in range(NF):
                ps = mktile(psp, [128, 512], F32, "mm_ps")
                for fc in range(NT):
                    nc.tensor.matmul(ps[:],
                                     c1T[fc][:, 128 * m:128 * (m + 1)],
                                     xnT[fc][:], start=(fc == 0),
                                     stop=(fc == NT - 1))
                o = mktile(sbp, [128, 512], BF16, f"relu_{m}")
                nc.vector.tensor_scalar(o[:], ps[:], 0.0, None, op0=ALU.max)
                relu.append(o)
            y2 = []
            for tc_ in range(NT):
                ps = mktile(psp, [128, 512], F32, "mm_ps")
                for m in range(NF):
                    nc.tensor.matmul(ps[:],
                                     relu[m][:, 128 * tc_:128 * (tc_ + 1)],
                                     c2T[m][:], start=(m == 0),
                                     stop=(m == NF - 1))
                o = mktile(sbp, [128, 512], F32, f"y2_{tc_}")
                nc.vector.tensor_tensor(r(o[:]), ps[:], xs[tc_][:],
                                        op=ALU.add)
                y2.append(o)

            # decomp2; trend head is emitted first so its serial
            # transpose/matmul chain overlaps the x-out subtracts and DMAs
            t2_ps = p.ma_matmul(psp, A_lhsT, y2)
            # trend = (t1 + t2) @ trend_w^T + trend_b  (t_i are the two
            # moving averages already computed for the decomps)
            az = []
            for c in range(NT):
                o = mktile(sbp, [128, 512], F32, f"az_{c}")
                nc.vector.tensor_tensor(o[:], t1_sb[c][:], t2_ps[c][:],
                                        op=ALU.add)
                az.append(o)
            for c in range(NT):
                o = mktile(sbp, [128, 512], F32, "x2_out", bufs=2)
                nc.vector.tensor_tensor(o[:], y2[c][:], t2_ps[c][:],
                                        op=ALU.subtract)
                nc.sync.dma_start(Hd["out_x"][128 * c:128 * (c + 1), :], o[:])
            with tc.tile_pool(name="trend_sb", bufs=1) as t3:
                azT = p.transpose_512(psp, t3, az, "azT")
                wtT = p.load_tiles(t3, "trend_wT")
                tb_row = p.rowload(t3, "trend_b_row")
                for tc_ in range(NT):
                    ps = mktile(psp, [128, 512], F32, "mm_ps")
                    nc.tensor.matmul(ps[:], r(p.ones1x128[:]), r(tb_row[:]),
                                     start=True, stop=False)
                    for fc in range(NT):
                        nc.tensor.matmul(
                            ps[:], r(azT[fc][:, 128 * tc_:128 * (tc_ + 1)]),
                            r(wtT[fc][:]), start=False, stop=(fc == NT - 1))
                    o = mktile(t3, [128, 512], F32, "tr_out", bufs=2)
                    nc.any.tensor_copy(o[:], ps[:])
                    nc.sync.dma_start(
                        Hd["out_trend"][128 * tc_:128 * (tc_ + 1), :], o[:])


def build_program(reps=1):
    nc = bacc.Bacc("TRN2", target_bir_lowering=False, debug=False)
    Hd = {}
    for name, shape, dt in SHARED_SPECS + PER_CORE_SPECS:
        Hd[name] = nc.dram_tensor(name, list(shape), dt, kind="ExternalInput")
    for name, shape, dt in OUT_SPECS:
        Hd[name] = nc.dram_tensor(name, list(shape), dt, kind="ExternalOutput")
    with tile.TileContext(nc) as tc:
        if reps > 1:
            with tc.For_i(0, reps, 1):
                emit(tc, nc, Hd)
        else:
            emit(tc, nc, Hd)
    nc.compile()
    return nc


# ----------------------------------------------------------------------------
# entry point
# ----------------------------------------------------------------------------

_LAST_EXEC_NS = None


def kernel(**inputs):
    global _LAST_EXEC_NS
    sh, per_core = host_prepare(inputs)
    nc = build_program()
    in_maps = []
    for b in range(B):
        m = dict(sh)
        m.update(per_core[b])
        in_maps.append(m)
    trace = os.environ.get("KBENCH_TRACE", "0") == "1"
    # the axon relay throws transient INTERNAL errors on a fraction of
    # executions; retry before giving up
    last_err = None
    for attempt in range(3):
        try:
            res = run_bass_kernel_spmd(nc, in_maps, list(range(B)),
                                       trace=trace)
            break
        except Exception as e:
            last_err = e
            print(f"[kernel] attempt {attempt} failed: {type(e).__name__}")
            import time as _time
            _time.sleep(2.0)
    else:
        raise last_err
    _LAST_EXEC_NS = res.exec_time_ns
    x = np.stack([res.results[b]["out_x"] for b in range(B)], axis=0)
    trend = np.stack([res.results[b]["out_trend"] for b in range(B)], axis=0)
    return np.stack([x, trend], axis=0)


# ----------------------------------------------------------------------------
# timing rig (test-only; the grading harness only calls kernel())
# ----------------------------------------------------------------------------

def _timed_fn(nc, in_maps):
    """jit fn over the donation path with device-resident inputs: per-call
    traffic is only the donated zero output buffers (~2MB/core)."""
    import jax
    from jax.sharding import Mesh, PartitionSpec, NamedSharding
    from jax.experimental.shard_map import shard_map
    from concourse import bass2jax
    from concourse.bass2jax import _bass_exec_p, install_neuronx_cc_hook

    install_neuronx_cc_hook()
    partition_name = nc.partition_id_tensor.name if nc.partition_id_tensor else None
    in_names, out_names, out_avals, zero_outs = [], [], [], []
    for alloc in nc.m.functions[0].allocations:
        if not isinstance(alloc, mybir.MemoryLocationSet):
            continue
        name = alloc.memorylocations[0].name
        if alloc.kind == "ExternalInput":
            if name != partition_name:
                in_names.append(name)
        elif alloc.kind == "ExternalOutput":
            out_names.append(name)
            shape = tuple(alloc.tensor_shape)
            dtype = mybir.dt.np(alloc.dtype)
            out_avals.append(jax.core.ShapedArray(shape, dtype))
            zero_outs.append(np.zeros(shape, dtype))
    n_params = len(in_names)
    n_outs = len(out_names)
    all_in = list(in_names) + list(out_names)
    if partition_name is not None:
        all_in.append(partition_name)

    def _body(*args):
        operands = list(args)
        if partition_name is not None:
            operands.append(bass2jax.partition_id_tensor())
        return tuple(_bass_exec_p.bind(
            *operands, out_avals=tuple(out_avals), in_names=tuple(all_in),
            out_names=tuple(out_names), lowering_input_output_aliases=(),
            sim_require_finite=True, sim_require_nnan=True, nc=nc))

    devices = jax.devices()[:B]
    mesh = Mesh(np.asarray(devices), ("core",))
    nio = n_params + n_outs
    fn = jax.jit(shard_map(_body, mesh=mesh,
                           in_specs=(PartitionSpec("core"),) * nio,
                           out_specs=(PartitionSpec("core"),) * n_outs,
                           check_rep=False),
                 donate_argnums=tuple(range(n_params, nio)),
                 keep_unused=True)
    concat_in = [np.concatenate([np.asarray(m[in_names[i]]) for m in in_maps],
                                axis=0) for i in range(n_params)]
    shard = NamedSharding(mesh, PartitionSpec("core"))
    dev_in = [jax.device_put(a, shard) for a in concat_in]
    jax.block_until_ready(dev_in)

    def mkouts():
        return [np.zeros((B * z.shape[0], *z.shape[1:]), z.dtype)
                for z in zero_outs]

    def call():
        import time
        a = mkouts()
        t0 = time.perf_counter()
        jax.block_until_ready(fn(*dev_in, *a))
        return time.perf_counter() - t0

    return call


def measure_exec_ns(inputs, rbig=65, n=8):
    """HW time per kernel execution, measured as the slope between a NEFF
    that runs the body once and one that runs it `rbig` times in a For_i
    hardware loop. Inputs are device-resident and outputs donated, so each
    timed call only ships the output buffers; interleaved sampling with
    min-statistics rejects the relay's latency noise."""
    sh, per_core = host_prepare(inputs)
    in_maps = []
    for b in range(B):
        m = dict(sh)
        m.update(per_core[b])
        in_maps.append(m)
    call1 = _timed_fn(build_program(reps=1), in_maps)
    callB = _timed_fn(build_program(reps=rbig), in_maps)
    w1s, wbs = [], []
    fails = 0
    for i in range(n + 2):
        if len(w1s) >= n:
            break
        try:
            if i == 0:
                call1()
                callB()   # warm both executables
                continue
            w1s.append(call1())
            wbs.append(callB())
        except Exception as e:
            fails += 1
            print(f"[timing] round {i} failed: {type(e).__name__}")
            if fails >= 3:
                break
    if not (w1s and wbs):
        return None
    print(f"[timing] w1(ms): {' '.join(f'{x * 1e3:.0f}' for x in w1s)}")
    print(f"[timing] w{rbig}(ms): {' '.join(f'{x * 1e3:.0f}' for x in wbs)}")
    per = (min(wbs) - min(w1s)) / (rbig - 1)
    return max(int(per * 1e9), 0)


# revision 42
# speedup vs baseline: 1.2616x; 1.2616x over previous
"""Trainium2 Bass kernel for nn_Model_15418932592810 (Autoformer-style decoder layer).

Sharding: data-parallel over batch B=8 across the 8 NeuronCores (one batch
per core, no collectives). Within a core, activations are kept feature-major
(D on partitions) through the matmul chains; attention is computed keys-major
(scores^T) so the softmax denominators ride along as a ones-column in V; the
series-decomp / layernorm tail runs token-major.
"""
import math
import os
import numpy as np
import ml_dtypes

import concourse.bass as bass
import concourse.mybir as mybir
import concourse.tile as tile
from concourse import bacc
from concourse.bass_utils import run_bass_kernel_spmd

F32 = mybir.dt.float32
F32R = mybir.dt.float32r
BF16 = mybir.dt.bfloat16
AX = mybir.AxisListType
ALU = mybir.AluOpType
ACTF = mybir.ActivationFunctionType

B, L, D, H, DH, DFF, KMA = 8, 512, 512, 8, 64, 2048, 25
NT = 4          # number of 128-row tiles in a 512 dim
NF = DFF // 128
EPS = 1e-5
BF = ml_dtypes.bfloat16


def r(x):
    return x.bitcast(F32R)


# ----------------------------------------------------------------------------
# host-side input preparation
# ----------------------------------------------------------------------------

def mktile(pool, shape, dtype, tag, bufs=None):
    return pool.tile(shape, dtype, name=tag, tag=tag, bufs=bufs)


def _softplus(x):
    return np.logaddexp(0.0, x.astype(np.float64))


def _ma_matrix():
    pad = (KMA - 1) // 2
    A = np.zeros((L, L), dtype=np.float64)
    for i in range(L):
        for m in range(i, i + KMA):
            j = min(max(m - pad, 0), L - 1)
            A[i, j] += 1.0 / KMA
    return A.astype(np.float32)


def _row(x):
    return np.ascontiguousarray(np.asarray(x, dtype=np.float32).reshape(1, -1))


def _cols(x):
    n = np.asarray(x).shape[0]
    return np.ascontiguousarray(np.asarray(x, np.float32).reshape(n // 128, 128).T)


def _T(w):
    return np.ascontiguousarray(np.asarray(w, dtype=np.float32).T)


def _pack(a):
    # (R, N) with R=128*c -> (128, c*N): column block c holds rows [128c,128c+128)
    a = np.asarray(a)
    rr, n = a.shape
    c = rr // 128
    return np.ascontiguousarray(
        a.reshape(c, 128, n).transpose(1, 0, 2).reshape(128, c * n))


def _Tp(w):
    return _pack(_T(w))


def host_prepare(inputs):
    ins = {k: np.asarray(v) for k, v in inputs.items()}
    sh = {}
    s = 1.0 / math.sqrt(DH)

    qkv_w = ins["sa_qkv_w"].astype(np.float64)
    qkv_b = ins["sa_qkv_b"].astype(np.float64)
    attn_sets = [
        ("sa", qkv_w[:D] * s, qkv_b[:D] * s, qkv_w[D:2 * D], qkv_b[D:2 * D],
         qkv_w[2 * D:], qkv_b[2 * D:], ins["sa_out_w"], ins["sa_out_b"]),
        ("cf", ins["cf_q_w"] * s, ins["cf_q_b"] * s, ins["cf_k_w"], ins["cf_k_b"],
         ins["cf_v_w"], ins["cf_v_b"], ins["cf_o_w"], ins["cf_o_b"]),
        ("cr", ins["cr_q_w"] * s, ins["cr_q_b"] * s, ins["cr_k_w"], ins["cr_k_b"],
         ins["cr_v_w"], ins["cr_v_b"], ins["cr_o_w"], ins["cr_o_b"]),
    ]
    # K bias is softmax-invariant (adds a per-query constant to every logit);
    # V bias is folded through the out projection (sum of attn weights == 1):
    # bo' = bo + Wo @ bv.
    for p, wq, bq, wk, bk, wv, bv, wo, bo in attn_sets:
        sh[f"{p}_wqT"] = _Tp(wq).astype(BF)
        sh[f"{p}_bq_c"] = _cols(bq)
        sh[f"{p}_wkT"] = _Tp(wk).astype(BF)
        sh[f"{p}_wvT"] = _Tp(wv).astype(BF)
        sh[f"{p}_woT"] = _Tp(wo).astype(BF)
        bo_eff = (np.asarray(bo, np.float64)
                  + np.asarray(wo, np.float64) @ np.asarray(bv, np.float64))
        sh[f"{p}_bo_c"] = _cols(bo_eff)

    wq, wk, wv = np.split(ins["mha_in_w"], 3, axis=0)
    bq, bk, bv = np.split(ins["mha_in_b"], 3, axis=0)
    sh["mha_wkT"] = _Tp(wk).astype(BF)
    sh["mha_wvT"] = _Tp(wv).astype(BF)
    qvec = ((ins["global_q"].reshape(D) @ wq.astype(np.float64).T + bq) * s
            ).astype(np.float32)
    # per-tile lhsT for the single-query scores: column h of block c holds
    # q chunk c masked to head h's 64-row half (heads 2c, 2c+1 live in tile c)
    qv2 = np.zeros((128, 4, 8), np.float32)
    for c in range(NT):
        qv2[0:64, c, 2 * c] = qvec[128 * c:128 * c + 64]
        qv2[64:128, c, 2 * c + 1] = qvec[128 * c + 64:128 * (c + 1)]
    sh["mha_qv2"] = np.ascontiguousarray(qv2.reshape(128, 32)).astype(BF)
    sh["mha_woT"] = _Tp(ins["mha_out_w"]).astype(BF)
    mha_bo_eff = (np.asarray(ins["mha_out_b"], np.float64)
                  + np.asarray(ins["mha_out_w"], np.float64)
                  @ np.asarray(bv, np.float64))
    sh["mha_bo_row"] = _row(mha_bo_eff)

    sh["fus_w1T"] = _Tp(ins["fusion_w"][:, :D]).astype(BF)
    sh["fus_w2T"] = _Tp(ins["fusion_w"][:, D:]).astype(BF)
    sh["fus_b_row"] = _row(ins["fusion_b"])

    # norm3 gamma folds into conv1 columns; normf/norm3 beta cancels in my_ln
    sh["conv1T"] = _Tp(ins["conv1_w"] * ins["norm3_g"][None, :]).astype(BF)
    sh["conv2T"] = _Tp(ins["conv2_w"]).astype(BF)
    sh["trend_wT"] = _Tp(ins["trend_w"])
    sh["trend_b_row"] = _row(ins["trend_b"])
    sh["gf_rep"] = np.ascontiguousarray(
        np.tile(np.asarray(ins["normf_g"], np.float32)[None, :], (128, 1)))

    i = np.arange(L, dtype=np.float64)
    rel = i[None, :] - i[:, None]                  # rel[q, k] = k - q
    lf = _softplus(ins["sa_lam_f"])[:, None, None]
    lb = _softplus(ins["sa_lam_b"])[:, None, None]
    decay = np.where(rel[None] < 0, -lb * np.abs(rel[None]),
                     np.where(rel[None] > 0, -lf * rel[None], 0.0))
    # stored as exp(decay): the bias is applied multiplicatively after the
    # exp (exp(l+b) == exp(l)*exp(b)), saving a PE matmul per score tile
    sh["sa_biasT"] = np.stack(
        [_pack(m) for m in np.exp(decay).transpose(0, 2, 1)]).astype(BF)

    # cos(2*pi*w*(q-k)) = cos(wq)cos(wk) + sin(wq)sin(wk): rank-2 per head.
    for p, lw in [("cf", "cf_logw"), ("cr", "cr_logw")]:
        w = np.exp(ins[lw].astype(np.float64))[:, None]
        ang = 2.0 * math.pi * w * i[None, :]       # (H, L)
        cs = np.stack([np.cos(ang), np.sin(ang)], axis=1)  # (H, 2, L)
        sh[f"{p}_cs"] = np.ascontiguousarray(
            cs.transpose(1, 0, 2).reshape(2, H * L)).astype(BF)

    sh["A_lhsT"] = _Tp(_ma_matrix())                # lhsT[j, i] = A[i, j]
    sh["identf"] = np.eye(128, dtype=np.float32)
    sh["identb"] = np.eye(128, dtype=BF)
    sh["ones_row128"] = np.ones((1, 128), np.float32)
    sh["ones_col128"] = np.ones((128, 1), np.float32)
    sh["eps_col128"] = np.full((128, 1), EPS, np.float32)

    per_core = []
    for b in range(B):
        per_core.append({
            "xsa_tok": _pack(ins["x_sa"][b].astype(np.float32)),
            "xsaT": _Tp(ins["x_sa"][b]).astype(BF),
            "xq1T": _Tp(ins["x_q1"][b]).astype(BF),
            "xq2T": _Tp(ins["x_q2"][b]).astype(BF),
        })
    return sh, per_core


# ----------------------------------------------------------------------------
# program builder
# ----------------------------------------------------------------------------

def _attn_specs(p):
    sp = [
        (f"{p}_wqT", (128, 2048), BF16), (f"{p}_bq_c", (128, 4), F32),
        (f"{p}_wkT", (128, 2048), BF16),
        (f"{p}_wvT", (128, 2048), BF16),
        (f"{p}_woT", (128, 2048), BF16), (f"{p}_bo_c", (128, 4), F32),
    ]
    if p == "sa":
        sp.append((f"{p}_biasT", (8, 128, 2048), BF16))
    else:
        sp.append((f"{p}_cs", (2, 4096), BF16))
    return sp


SHARED_SPECS = (
    _attn_specs("sa") + _attn_specs("cf") + _attn_specs("cr") + [
        ("mha_wkT", (128, 2048), BF16),
        ("mha_wvT", (128, 2048), BF16),
        ("mha_qv2", (128, 32), BF16),
        ("mha_woT", (128, 2048), BF16), ("mha_bo_row", (1, 512), F32),
        ("fus_w1T", (128, 2048), BF16), ("fus_w2T", (128, 2048), BF16),
        ("fus_b_row", (1, 512), F32),
        ("conv1T", (128, 8192), BF16), ("conv2T", (128, 8192), BF16),
        ("trend_wT", (128, 2048), F32), ("trend_b_row", (1, 512), F32),
        ("gf_rep", (128, 512), F32),
        ("A_lhsT", (128, 2048), F32),
        ("identf", (128, 128), F32), ("identb", (128, 128), BF16),
        ("ones_row128", (1, 128), F32), ("ones_col128", (128, 1), F32),
        ("eps_col128", (128, 1), F32),
    ]
)
PER_CORE_SPECS = [
    ("xsa_tok", (128, 2048), F32), ("xsaT", (128, 2048), BF16),
    ("xq1T", (128, 2048), BF16), ("xq2T", (128, 2048), BF16),
]
OUT_SPECS = [("out_x", (512, 512), F32), ("out_trend", (512, 512), F32)]
DTYPES = {name: dt for name, _, dt in SHARED_SPECS + PER_CORE_SPECS}


def hslice(tiles, h):
    t = tiles[h // 2]
    off = 64 * (h % 2)
    return t[off:off + 64, :]


class Prog:
    def __init__(self, nc, tc, Hd):
        self.nc, self.tc, self.Hd = nc, tc, Hd

    def load_tiles(self, pool, name, n=NT, width=512, dtype=None, tag=None,
                   bufs=None, split=False):
        if dtype is None:
            dtype = DTYPES.get(name, F32)
        """One packed DMA (or n chunked DMAs when split=True, so the first
        consumer can start before the whole tensor lands); returns the n
        (128, width) chunk views."""
        nc = self.nc
        dram = self.Hd[name]
        t = mktile(pool, [128, n * width], dtype, tag or name, bufs=bufs)
        if split:
            for c in range(n):
                sl = slice(width * c, width * (c + 1))
                if dtype == F32:
                    nc.sync.dma_start(r(t[:, sl]), r(dram[:, sl]))
                else:
                    nc.sync.dma_start(t[:, sl], dram[:, sl])
        elif dtype == F32:
            nc.sync.dma_start(r(t[:]), r(dram[:]))
        else:
            nc.sync.dma_start(t[:], dram[:])
        return [t[:, width * c:width * (c + 1)] for c in range(n)]

    def rowload(self, pool, name, width=512, tag=None, bufs=None):
        t = mktile(pool, [1, width], F32, tag or name, bufs=bufs)
        self.nc.sync.dma_start(r(t[:]), r(self.Hd[name][:]))
        return t

    def colload(self, pool, name, n=NT, tag=None, bufs=None):
        t = mktile(pool, [128, n], F32, tag or name, bufs=bufs)
        self.nc.sync.dma_start(r(t[:]), r(self.Hd[name][:]))
        return t

    # ------------------------------------------------------------------
    def proj_fm(self, psp, sbp, xT, wT_sb, b_c, tag):
        """Feature-major projection: out^T = W @ x^T + b, NT tiles (128,512).
        bf16 in, bf16 out."""
        nc = self.nc
        n_out = wT_sb[0].shape[1]
        outs = []
        for m in range(n_out // 128):
            ps = mktile(psp, [128, 512], F32, "mm_ps")
            for kc in range(NT):
                nc.tensor.matmul(ps[:], wT_sb[kc][:, 128 * m:128 * (m + 1)],
                                 xT[kc][:], start=(kc == 0), stop=(kc == NT - 1))
            o = mktile(sbp, [128, 512], BF16, f"{tag}_{m}")
            if b_c is None:
                nc.any.tensor_copy(o[:], ps[:])
            else:
                nc.vector.tensor_scalar(o[:], ps[:], b_c[:, m:m + 1], None,
                                        op0=ALU.add)
            outs.append(o)
        return outs

    def proj_tok_aug(self, psp, sbp, xT, wvT_sb, tag):
        """Token-major V projection with a ones column interleaved per head.
        The V bias is folded into the out-projection bias host-side."""
        nc = self.nc
        outs = []
        for kc in range(NT):
            ps = mktile(psp, [128, 512], F32, "mm_ps")
            for fc in range(NT):
                nc.tensor.matmul(ps[:], xT[fc][:, 128 * kc:128 * (kc + 1)],
                                 wvT_sb[fc][:], start=(fc == 0),
                                 stop=(fc == NT - 1))
            o = mktile(sbp, [128, 520], BF16, f"{tag}_{kc}")
            nc.vector.memset(o[:], 1.0)
            ov = o[:].rearrange("p (h c) -> p h c", c=65)
            nc.vector.tensor_copy(
                ov[:, :, 0:64], ps[:].rearrange("p (h c) -> p h c", c=64))
            outs.append(o)
        return outs

    def attention(self, name, qin, kvin, resid_q=False, preq_w=None):
        """8-head attention with additive bias; returns feature-major NT tiles
        allocated in the persistent pool. qin/kvin: DRAM name or tile list."""
        nc, tc, Hd = self.nc, self.tc, self.Hd
        outs = [mktile(self.aopool, [128, 512], BF16, f"{name}_out_{m}")
                for m in range(NT)]
        with (
            tc.tile_pool(name=f"{name}_ps", bufs=2, space="PSUM") as psp,
            tc.tile_pool(name=f"{name}_sb", bufs=1) as sbp,
        ):
            wp = self.wpool
            if isinstance(qin, str):
                qinT = self.load_tiles(wp, qin, tag="at_inq", bufs=2)
            else:
                qinT = qin
            if preq_w is None:
                wqT = self.load_tiles(wp, f"{name}_wqT", tag="at_wq", bufs=2)
            else:
                wqT = preq_w
            residT = qinT if resid_q else None
            bq_c = self.colload(wp, f"{name}_bq_c", tag="at_bq", bufs=2)
            if isinstance(kvin, str):
                kvinT = (qinT if kvin == qin else
                         self.load_tiles(wp, kvin, tag="at_inkv", bufs=2))
            else:
                kvinT = kvin
            wkT = self.load_tiles(wp, f"{name}_wkT", tag="at_wk", bufs=2)
            wvT = self.load_tiles(wp, f"{name}_wvT", tag="at_wv", bufs=2)
            woT = self.load_tiles(wp, f"{name}_woT", tag="at_wo", bufs=1)
            bo_c = self.colload(wp, f"{name}_bo_c", tag="at_bo", bufs=2)
            if name == "sa":
                biasT = Hd[f"{name}_biasT"]
                cs = None
            else:
                biasT = None
                cs = mktile(sbp, [2, 4096], BF16, "at_cs")
                nc.sync.dma_start(cs[:], Hd[f"{name}_cs"][:])

            qT = self.proj_fm(psp, sbp, qinT, wqT, bq_c, "at_q")
            kT = self.proj_fm(psp, sbp, kvinT, wkT, None, "at_k")
            vaug = self.proj_tok_aug(psp, sbp, kvinT, wvT, "at_v")

            aout = []
            for half in range(2):
                expT = [mktile(sbp, [128, 2048], BF16, f"at_exp_{kc}", bufs=1)
                        for kc in range(NT)]
                for h4 in range(4):
                    h = 4 * half + h4
                    if biasT is not None:
                        bsl = mktile(sbp, [128, 2048], BF16, "at_bias", bufs=2)
                        nc.sync.dma_start(bsl[:], biasT[h])
                    for kc in range(NT):
                        ps = mktile(psp, [128, 512], F32, "sc_ps")
                        if biasT is None:
                            nc.tensor.matmul(
                                ps[:],
                                cs[:, 512 * h + 128 * kc:
                                     512 * h + 128 * (kc + 1)],
                                cs[:, 512 * h:512 * (h + 1)],
                                start=True, stop=False)
                        nc.tensor.matmul(
                            ps[:],
                            hslice(kT, h)[:, 128 * kc:128 * (kc + 1)],
                            hslice(qT, h), start=(biasT is not None),
                            stop=True)
                        esl = expT[kc][:, 512 * h4:512 * (h4 + 1)]
                        nc.scalar.activation(esl, ps[:], ACTF.Exp)
                        if biasT is not None:
                            nc.vector.tensor_tensor(
                                esl, esl, bsl[:, 512 * kc:512 * (kc + 1)],
                                op=ALU.mult)
                # AV with the softmax denominator riding as the 65th row
                # per head. Each h4 group gets its own PSUM bank so the next
                # group's matmuls don't serialize against this group's
                # reciprocal/broadcast/normalize readers.
                for h4 in range(4):
                    h = 4 * half + h4
                    av = mktile(psp, [65, 512], F32, "av", bufs=4)
                    for kc in range(NT):
                        nc.tensor.matmul(av[:],
                                         vaug[kc][:, 65 * h:65 * h + 65],
                                         expT[kc][:, 512 * h4:512 * (h4 + 1)],
                                         start=(kc == 0), stop=(kc == NT - 1))
                    recip = mktile(sbp, [1, 512], F32, "at_recip", bufs=2)
                    nc.vector.reciprocal(recip[:], av[64:65, :])
                    rep = mktile(sbp, [64, 512], F32, "at_rep", bufs=2)
                    nc.gpsimd.partition_broadcast(rep[:], recip[:])
                    if h % 2 == 0:
                        pair = mktile(sbp, [128, 512], BF16, f"at_ao_{h // 2}")
                        aout.append(pair)
                    off = 64 * (h % 2)
                    nc.vector.tensor_tensor(aout[h // 2][off:off + 64, :],
                                            av[0:64, :], rep[:], op=ALU.mult)

            for m in range(NT):
                ps = mktile(psp, [128, 512], F32, "mm_ps")
                for c in range(NT):
                    nc.tensor.matmul(ps[:],
                                     woT[c][:, 128 * m:128 * (m + 1)],
                                     aout[c][:], start=(c == 0),
                                     stop=(c == NT - 1))
                nc.vector.tensor_scalar(outs[m][:], ps[:],
                                        bo_c[:, m:m + 1], None, op0=ALU.add)
                if residT is not None:
                    nc.vector.tensor_tensor(outs[m][:], outs[m][:],
                                            residT[m][:], op=ALU.add)
        return outs

    # ------------------------------------------------------------------
    def ctx_pool_fusion(self, ca1outT, sa_outT, wkT, qv2, xsa_tok):
        """MultiheadAttention pooling with precomputed 1-token query.

        Single-query attention is computed head-major: scores land as an
        (8, 512) tile (one head per partition) via per-chunk masked-q lhsT,
        so exp/sum/normalize all run on the right axes with no broadcasts.
        The fusion W1 matmuls are emitted mid-chain to keep PE busy during
        the exp/transpose latency. Returns (s2b row, fusion tiles)."""
        nc, tc = self.nc, self.tc
        s2b = mktile(self.keep, [1, 512], F32, "s2b_row")
        f_tiles = [mktile(self.keep, [128, 512], F32, f"fus_{c}")
                   for c in range(NT)]
        with (
            tc.tile_pool(name="mha_ps", bufs=2, space="PSUM") as psp,
            tc.tile_pool(name="mha_sb", bufs=1) as sbp,
        ):
            wvT = self.load_tiles(sbp, "mha_wvT")
            w1T = self.load_tiles(sbp, "fus_w1T")
            woT = self.load_tiles(sbp, "mha_woT")
            bo_row = self.rowload(sbp, "mha_bo_row")
            w2T = self.load_tiles(sbp, "fus_w2T")
            fus_b_row = self.rowload(sbp, "fus_b_row")

            kT = self.proj_fm(psp, sbp, ca1outT, wkT, None, "mha_k")

            sc_ps = mktile(psp, [8, 512], F32, "sc_y_ps", bufs=1)
            for c in range(NT):
                nc.tensor.matmul(sc_ps[:], qv2[:, 8 * c:8 * (c + 1)],
                                 kT[c][:], start=(c == 0),
                                 stop=(c == NT - 1))
            e = mktile(sbp, [8, 512], F32, "mha_e")
            zsum = mktile(sbp, [8, 1], F32, "mha_z")
            nc.scalar.activation(e[:], sc_ps[:], ACTF.Exp, accum_out=zsum[:])
            rz = mktile(sbp, [8, 1], F32, "mha_rz")
            nc.vector.reciprocal(rz[:], zsum[:])

            # V projection fills PE while the exp/transpose chain drains
            v_tok = []
            for kc in range(NT):
                ps = mktile(psp, [128, 512], F32, "mm_ps")
                for fc in range(NT):
                    nc.tensor.matmul(ps[:],
                                     ca1outT[fc][:, 128 * kc:128 * (kc + 1)],
                                     wvT[fc][:], start=(fc == 0),
                                     stop=(fc == NT - 1))
                o = mktile(sbp, [128, 512], BF16, f"mha_v_{kc}")
                nc.vector.tensor_copy(o[:], ps[:])
                v_tok.append(o)

            t_ps = mktile(psp, [128, 32], F32, "mha_t_ps", bufs=1)
            for c in range(NT):
                nc.tensor.transpose(t_ps[:, 8 * c:8 * (c + 1)],
                                    e[0:8, 128 * c:128 * (c + 1)],
                                    self.identf[0:8, 0:8])
            attnT = mktile(sbp, [128, 32], BF16, "mha_attnT")
            nc.vector.tensor_copy(attnT[:], t_ps[:])
            y_ps = mktile(psp, [8, 512], F32, "sc_y_ps", bufs=1)
            for c in range(NT):
                nc.tensor.matmul(y_ps[:], attnT[:, 8 * c:8 * (c + 1)],
                                 v_tok[c][:], start=(c == 0),
                                 stop=(c == NT - 1))
            y_sb = mktile(sbp, [8, 512], F32, "mha_y")
            nc.vector.tensor_scalar(y_sb[:], y_ps[:], rz[:, 0:1], None,
                                    op0=ALU.mult)

            # gather y into feature-major (head h's 64-dim block of row h)
            t2 = mktile(psp, [128, 32], F32, "mha_t_ps", bufs=1)
            for c in range(NT):
                nc.tensor.transpose(t2[:, 8 * c:8 * (c + 1)],
                                    y_sb[0:8, 128 * c:128 * (c + 1)],
                                    self.identf[0:8, 0:8])
            y_fm = mktile(sbp, [128, NT], BF16, "mha_yfm")
            for c in range(NT):
                nc.vector.tensor_copy(
                    y_fm[0:64, c:c + 1],
                    t2[0:64, 8 * c + 2 * c:8 * c + 2 * c + 1])
                nc.vector.tensor_copy(
                    y_fm[64:128, c:c + 1],
                    t2[64:128, 8 * c + 2 * c + 1:8 * c + 2 * c + 2])

            s_ps = mktile(psp, [1, 512], F32, "mha_s_ps", bufs=1)
            nc.tensor.matmul(s_ps[:], r(self.ones1x1[:]), r(bo_row[:]),
                             start=True, stop=False)
            for c in range(NT):
                nc.tensor.matmul(s_ps[:], y_fm[:, c:c + 1], woT[c][:],
                                 start=False, stop=(c == NT - 1))
            s_row = mktile(sbp, [1, 512], F32, "mha_srow")
            nc.vector.tensor_copy(s_row[:], s_ps[:])
            t3 = mktile(psp, [128, 32], F32, "mha_t_ps", bufs=1)
            for c in range(NT):
                nc.tensor.transpose(t3[:, c:c + 1],
                                    s_row[0:1, 128 * c:128 * (c + 1)],
                                    self.identf[0:1, 0:1])
            s_fm = mktile(sbp, [128, NT], BF16, "mha_sfm")
            nc.vector.tensor_copy(s_fm[:], t3[:, 0:NT])
            s2_ps = mktile(psp, [1, 512], F32, "mha_s_ps", bufs=1)
            nc.tensor.matmul(s2_ps[:], r(self.ones1x1[:]), r(fus_b_row[:]),
                             start=True, stop=False)
            for c in range(NT):
                nc.tensor.matmul(s2_ps[:], s_fm[:, c:c + 1], w2T[c][:],
                                 start=False, stop=(c == NT - 1))
            nc.vector.tensor_copy(r(s2b[:]), s2_ps[:])

            # fusion W1 matmuls, with the x_sa residual (identity matmul)
            # and the ctx-pool row (K=1 ones broadcast) accumulated in PSUM:
            # the tiles coming out are the finished pre-normf activations
            for tc_ in range(NT):
                ps = mktile(psp, [128, 512], F32, "mm_ps")
                nc.tensor.matmul(ps[:], r(self.identf[:]),
                                 r(xsa_tok[tc_][:]), start=True, stop=False)
                nc.tensor.matmul(ps[:], r(self.ones1x128[:]), r(s2b[:]),
                                 start=False, stop=False)
                for fc in range(NT):
                    nc.tensor.matmul(
                        ps[:], sa_outT[fc][:, 128 * tc_:128 * (tc_ + 1)],
                        w1T[fc][:], start=False, stop=(fc == NT - 1))
                if tc_ % 2:
                    nc.any.tensor_copy(r(f_tiles[tc_][:]), ps[:])
                else:
                    nc.vector.tensor_copy(r(f_tiles[tc_][:]), ps[:])
        return s2b, f_tiles

    # ------------------------------------------------------------------
    def my_ln(self, psp, sbp, out_pool, y_tiles, tag, gmul=None, outs=None):
        """Full my_Layernorm: per-token stats + normalize, then subtract the
        per-sequence mean. Everything is per-tile pipelined: tile c flows
        stats -> normalize -> seq-sum matmul while tile c+1 computes stats,
        so the serial chain exposed to PE is ~one tile, not four."""
        nc = self.nc
        ps = mktile(psp, [1, 512], F32, "sm_ps", bufs=1)
        xh = []
        for c in range(NT):
            sums = mktile(sbp, [128, 1], F32, f"{tag}_sums", bufs=4)
            nc.vector.tensor_reduce(sums[:], y_tiles[c][:], axis=AX.X,
                                    op=ALU.add)
            sq = mktile(sbp, [128, 512], F32, "lnsq", bufs=2)
            sumsq = mktile(sbp, [128, 1], F32, f"{tag}_sumsq", bufs=4)
            nc.scalar.activation(sq[:], y_tiles[c][:], ACTF.Square,
                                 accum_out=sumsq[:])
            mu = mktile(sbp, [128, 1], F32, f"{tag}_mu", bufs=4)
            nc.vector.tensor_scalar(mu[:], sums[:], 1.0 / D, None,
                                    op0=ALU.mult)
            var = mktile(sbp, [128, 1], F32, f"{tag}_var", bufs=4)
            nc.vector.tensor_tensor(var[:], mu[:], mu[:], op=ALU.mult)
            msq = mktile(sbp, [128, 1], F32, f"{tag}_msq", bufs=4)
            nc.vector.tensor_scalar(msq[:], sumsq[:], 1.0 / D, None,
                                    op0=ALU.mult)
            nc.vector.tensor_tensor(var[:], msq[:], var[:], op=ALU.subtract)
            std = mktile(sbp, [128, 1], F32, f"{tag}_std", bufs=4)
            nc.scalar.activation(std[:], var[:], ACTF.Sqrt,
                                 bias=self.eps_col[:, 0:1])
            rstd = mktile(sbp, [128, 1], F32, f"{tag}_rstd", bufs=4)
            nc.vector.reciprocal(rstd[:], std[:])
            o = mktile(sbp, [128, 512], F32, "ln_xh", bufs=4)
            nc.vector.tensor_scalar(r(o[:]), y_tiles[c][:], mu[:],
                                    rstd[:], op0=ALU.subtract, op1=ALU.mult)
            nc.tensor.matmul(ps[:], r(self.ones_col[:, 0:1]), r(o[:]),
                             start=(c == 0), stop=(c == NT - 1))
            xh.append(o)
        row = mktile(sbp, [1, 512], F32, f"{tag}_row")
        nc.scalar.mul(r(row[:]), ps[:], 1.0 / L)
        rep_ps = mktile(psp, [128, 512], F32, "bcast_ps", bufs=1)
        nc.tensor.matmul(rep_ps[:], r(self.ones1x128[:]), r(row[:]),
                         start=True, stop=True)
        if outs is None:
            outs = [mktile(out_pool, [128, 512], F32, f"{tag}_out_{c}")
                    for c in range(NT)]
        for c in range(NT):
            nc.vector.tensor_tensor(r(outs[c][:]), xh[c][:], rep_ps[:],
                                    op=ALU.subtract)
            if gmul is not None:
                eng = nc.gpsimd if c % 2 else nc.vector
                eng.tensor_tensor(r(outs[c][:]), outs[c][:], gmul[:],
                                  op=ALU.mult)
        return outs

    def seqmean_sub(self, psp, sbp, out_pool, xh_tiles, tag, gmul=None,
                    outs=None):
        """Subtract the per-sequence (partition-axis) mean. The mean row is
        replicated to 128 partitions with a K=1 PE matmul into PSUM (~0.2us)
        instead of a gpsimd partition_broadcast (~3us)."""
        nc = self.nc
        ps = mktile(psp, [1, 512], F32, "sm_ps", bufs=1)
        for c in range(NT):
            nc.tensor.matmul(ps[:], r(self.ones_col[:, 0:1]), r(xh_tiles[c][:]),
                             start=(c == 0), stop=(c == NT - 1))
        row = mktile(sbp, [1, 512], F32, f"{tag}_row")
        nc.scalar.mul(row[:], ps[:], 1.0 / L)
        rep_ps = mktile(psp, [128, 512], F32, "bcast_ps", bufs=1)
        nc.tensor.matmul(rep_ps[:], r(self.ones1x128[:]), r(row[:]),
                         start=True, stop=True)
        if outs is None:
            outs = [mktile(out_pool, [128, 512], F32, f"{tag}_out_{c}")
                    for c in range(NT)]
        for c in range(NT):
            nc.vector.tensor_tensor(r(outs[c][:]), xh_tiles[c][:], rep_ps[:],
                                    op=ALU.subtract)
            if gmul is not None:
                nc.vector.tensor_tensor(r(outs[c][:]), outs[c][:], gmul[:],
                                        op=ALU.mult)
        return outs

    def transpose_512(self, psp, sbp, src_tiles, tag):
        nc = self.nc
        outs = [mktile(sbp, [128, 512], F32, f"{tag}_{c}") for c in range(NT)]
        for rr in range(NT):
            for cc in range(NT):
                ps = mktile(psp, [128, 128], F32, "tr_ps")
                nc.tensor.transpose(ps[:],
                                    src_tiles[rr][:, 128 * cc:128 * (cc + 1)],
                                    self.identf[:])
                nc.any.tensor_copy(r(outs[cc][:, 128 * rr:128 * (rr + 1)]),
                                   ps[:])
        return outs

    def ma_matmul(self, psp, A_lhsT, x_tiles):
        """A is banded (half-bandwidth 12), so tile block (tc, kc) is zero
        unless |tc - kc| <= 1: 10 matmuls instead of 16."""
        nc = self.nc
        pss = []
        for tc_ in range(NT):
            kcs = [kc for kc in range(NT) if abs(kc - tc_) <= 1]
            ps = mktile(psp, [128, 512], F32, "ma_ps", bufs=2)
            for i, kc in enumerate(kcs):
                nc.tensor.matmul(ps[:],
                                 r(A_lhsT[kc][:, 128 * tc_:128 * (tc_ + 1)]),
                                 r(x_tiles[kc][:]), start=(i == 0),
                                 stop=(i == len(kcs) - 1))
            pss.append(ps)
        return pss


def emit(tc, nc, Hd):
    p = Prog(nc, tc, Hd)
    with tc.tile_pool(name="keep", bufs=1) as keep:
        p.keep = keep
        # three attention blocks; their outputs live in a scoped pool that
        # closes before the tail pools open (frees 24KB/partition)
        ao_cm = tc.tile_pool(name="ao", bufs=1)
        p.aopool = ao_cm.__enter__()
        ctxw_cm = tc.tile_pool(name="ctxw", bufs=1)
        ctxw = ctxw_cm.__enter__()
        with tc.tile_pool(name="wts", bufs=2) as wpool:
            p.wpool = wpool
            # the first projection's operands head the DMA queue; constants
            # (needed only ~15us in) follow them
            sa_qt = mktile(wpool, [128, 2048], BF16, "at_inq", bufs=2)
            sa_wt = mktile(wpool, [128, 2048], BF16, "at_wq", bufs=2)
            for c in range(NT):
                sl = slice(512 * c, 512 * (c + 1))
                nc.sync.dma_start(sa_wt[:, sl], Hd["sa_wqT"][:, sl])
                nc.sync.dma_start(sa_qt[:, sl], Hd["xsaT"][:, sl])
            sa_qinT = [sa_qt[:, 512 * c:512 * (c + 1)] for c in range(NT)]
            sa_wqT = [sa_wt[:, 512 * c:512 * (c + 1)] for c in range(NT)]
            p.ones1x128 = mktile(keep, [1, 128], F32, "ones1x128")
            nc.sync.dma_start(r(p.ones1x128[:]), r(Hd["ones_row128"][:]))
            p.ones1x1 = p.ones1x128[0:1, 0:1]
            p.ones_col = mktile(keep, [128, 1], F32, "ones_col")
            nc.sync.dma_start(r(p.ones_col[:]), r(Hd["ones_col128"][:]))
            p.eps_col = mktile(keep, [128, 1], F32, "eps_col")
            nc.sync.dma_start(p.eps_col[:], Hd["eps_col128"][:])
            p.identf = mktile(keep, [128, 128], F32, "identf")
            nc.sync.dma_start(r(p.identf[:]), r(Hd["identf"][:]))
            sa_outT = p.attention("sa", sa_qinT, sa_qinT, preq_w=sa_wqT)
            xq1T = p.load_tiles(keep, "xq1T")
            xsa_tok = p.load_tiles(keep, "xsa_tok")
            enrichedT = p.attention("cf", "xq2T", xq1T, resid_q=True)
            # prefetch the ctx-pool K weights + query so kproj can start the
            # moment the cr out-projection lands
            mha_wkT = p.load_tiles(ctxw, "mha_wkT")
            mha_qv2 = mktile(ctxw, [128, 32], BF16, "mha_qv2")
            nc.sync.dma_start(mha_qv2[:], Hd["mha_qv2"][:])
            ca1outT = p.attention("cr", xq1T, enrichedT)

        # context pooling + fusion W1 matmuls (interleaved inside)
        s2b, f_tiles = p.ctx_pool_fusion(ca1outT, sa_outT,
                                         mha_wkT, mha_qv2, xsa_tok)
        ctxw_cm.__exit__(None, None, None)
        ao_cm.__exit__(None, None, None)

        # persistent tail tensors
        xln = [mktile(keep, [128, 512], F32, f"xln_{c}") for c in range(NT)]
        xs = [mktile(keep, [128, 512], F32, f"xs_{c}") for c in range(NT)]

        with (
            tc.tile_pool(name="tail_ps", bufs=2, space="PSUM") as psp,
            tc.tile_pool(name="tail_sb", bufs=1) as sbp,
        ):
            gf_rep = mktile(sbp, [128, 512], F32, "gf_rep")
            nc.sync.dma_start(gf_rep[:], Hd["gf_rep"][:])
            A_lhsT = p.load_tiles(sbp, "A_lhsT")
            # conv weights (bf16) land while the pre-FFN tail chain runs
            c1T = p.load_tiles(sbp, "conv1T", width=2048, dtype=BF16)
            c2T = p.load_tiles(sbp, "conv2T", n=NF, dtype=BF16)

            y_tiles = f_tiles

            # normf (token-major my_ln); beta cancels, gamma applied
            p.my_ln(psp, sbp, sbp, y_tiles, "lnf", gmul=gf_rep, outs=xln)

            # decomp1: xs = xln - A @ xln; keep the trend part for the
            # trend head (A@(xln+y2) == t1 + t2 by linearity)
            t1_ps = p.ma_matmul(psp, A_lhsT, xln)
            t1_sb = []
            for c in range(NT):
                nc.vector.tensor_tensor(xs[c][:], xln[c][:], t1_ps[c][:],
                                        op=ALU.subtract)
                t1c = mktile(sbp, [128, 512], F32, f"t1_{c}")
                nc.any.tensor_copy(t1c[:], t1_ps[c][:])
                t1_sb.append(t1c)

            # norm3 (gamma folded into conv1, beta cancels)
            xn = p.my_ln(psp, sbp, sbp, xs, "ln3")

            # ffn: transpose -> conv1+relu (feature-major) -> conv2 (token-major)
            # conv weights are bf16 and were prefetched in the fw pool.
            relu = []
            xnT = []
            for c in range(NT):
                t = mktile(sbp, [128, 512], BF16, f"xnT_{c}")
                xnT.append(t)
            for rr in range(NT):
                for cc in range(NT):
                    tp = mktile(psp, [128, 128], F32, "tr_ps")
                    nc.tensor.transpose(tp[:],
                                        xn[rr][:, 128 * cc:128 * (cc + 1)],
                                        p.identf[:])
                    nc.any.tensor_copy(xnT[cc][:, 128 * rr:128 * (rr + 1)],
                                       tp[:])
            for m in range(NF):
                ps = mktile(psp, [128, 512], F32, "mm_ps")
                for fc in range(NT):
                    nc.tensor.matmul(ps[:],
                                     c1T[fc][:, 128 * m:128 * (m + 1)],
                                     xnT[fc][:], start=(fc == 0),
                                     stop=(fc == NT - 1))
                o = mktile(sbp, [128, 512], BF16, f"relu_{m}")
                nc.vector.tensor_scalar(o[:], ps[:], 0.0, None, op0=ALU.max)
                relu.append(o)
            y2 = []
            for tc_ in range(NT):
                ps = mktile(psp, [128, 512], F32, "mm_ps")
                for m in range(NF):
                    nc.tensor.matmul(ps[:],
                                     relu[m][:, 128 * tc_:128 * (tc_ + 1)],
                                     c2T[m][:], start=(m == 0),
                                     stop=(m == NF - 1))
                o = mktile(sbp, [128, 512], F32, f"y2_{tc_}")
                nc.vector.tensor_tensor(r(o[:]), ps[:], xs[tc_][:],
                                        op=ALU.add)
                y2.append(o)

            # decomp2; trend head is emitted first so its serial
            # transpose/matmul chain overlaps the x-out subtracts and DMAs
            t2_ps = p.ma_matmul(psp, A_lhsT, y2)
            # trend = (t1 + t2) @ trend_w^T + trend_b  (t_i are the two
            # moving averages already computed for the decomps)
            az = []
            for c in range(NT):
                o = mktile(sbp, [128, 512], F32, f"az_{c}")
                nc.vector.tensor_tensor(o[:], t1_sb[c][:], t2_ps[c][:],
                                        op=ALU.add)
                az.append(o)
            for c in range(NT):
                o = mktile(sbp, [128, 512], F32, "x2_out", bufs=2)
                nc.vector.tensor_tensor(o[:], y2[c][:], t2_ps[c][:],
                                        op=ALU.subtract)
                nc.sync.dma_start(Hd["out_x"][128 * c:128 * (c + 1), :], o[:])
            with tc.tile_pool(name="trend_sb", bufs=1) as t3:
                azT = p.transpose_512(psp, t3, az, "azT")
                wtT = p.load_tiles(t3, "trend_wT")
                tb_row = p.rowload(t3, "trend_b_row")
                for tc_ in range(NT):
                    ps = mktile(psp, [128, 512], F32, "mm_ps")
                    nc.tensor.matmul(ps[:], r(p.ones1x128[:]), r(tb_row[:]),
                                     start=True, stop=False)
                    for fc in range(NT):
                        nc.tensor.matmul(
                            ps[:], r(azT[fc][:, 128 * tc_:128 * (tc_ + 1)]),
                            r(wtT[fc][:]), start=False, stop=(fc == NT - 1))
                    o = mktile(t3, [128, 512], F32, "tr_out", bufs=2)
                    nc.any.tensor_copy(o[:], ps[:])
                    nc.sync.dma_start(
                        Hd["out_trend"][128 * tc_:128 * (tc_ + 1), :], o[:])


def build_program(reps=1):
    nc = bacc.Bacc("TRN2", target_bir_lowering=False, debug=False)
    Hd = {}
    for name, shape, dt in SHARED_SPECS + PER_CORE_SPECS:
        Hd[name] = nc.dram_tensor(name, list(shape), dt, kind="ExternalInput")
    for name, shape, dt in OUT_SPECS:
        Hd[name] = nc.dram_tensor(name, list(shape), dt, kind="ExternalOutput")
    with tile.TileContext(nc) as tc:
        if reps > 1:
            with tc.For_i(0, reps, 1):
                emit(tc, nc, Hd)
        else:
            emit(tc, nc, Hd)
    nc.compile()
    return nc


# ----------------------------------------------------------------------------
# entry point
# ----------------------------------------------------------------------------

_LAST_EXEC_NS = None


def kernel(**inputs):
    global _LAST_EXEC_NS
    sh, per_core = host_prepare(inputs)
    nc = build_program()
    in_maps = []
    for b in range(B):
        m = dict(sh)
        m.update(per_core[b])
        in_maps.append(m)
    trace = os.environ.get("KBENCH_TRACE", "0") == "1"
    # the axon relay throws transient INTERNAL errors on a fraction of
    # executions; retry before giving up
    last_err = None
    for attempt in range(3):
        try:
            res = run_bass_kernel_spmd(nc, in_maps, list(range(B)),
                                       trace=trace)
            break
        except Exception as e:
            last_err = e
            print(f"[kernel] attempt {attempt} failed: {type(e).__name__}")
            import time as _time
            _time.sleep(2.0)
    else:
        raise last_err
    _LAST_EXEC_NS = res.exec_time_ns
    x = np.stack([res.results[b]["out_x"] for b in range(B)], axis=0)
    trend = np.stack([res.results[b]["out_trend"] for b in range(B)], axis=0)
    return np.stack([x, trend], axis=0)


# ----------------------------------------------------------------------------
# timing rig (test-only; the grading harness only calls kernel())
# ----------------------------------------------------------------------------

def _timed_fn(nc, in_maps):
    """jit fn over the donation path with device-resident inputs: per-call
    traffic is only the donated zero output buffers (~2MB/core)."""
    import jax
    from jax.sharding import Mesh, PartitionSpec, NamedSharding
    from jax.experimental.shard_map import shard_map
    from concourse import bass2jax
    from concourse.bass2jax import _bass_exec_p, install_neuronx_cc_hook

    install_neuronx_cc_hook()
    partition_name = nc.partition_id_tensor.name if nc.partition_id_tensor else None
    in_names, out_names, out_avals, zero_outs = [], [], [], []
    for alloc in nc.m.functions[0].allocations:
        if not isinstance(alloc, mybir.MemoryLocationSet):
            continue
        name = alloc.memorylocations[0].name
        if alloc.kind == "ExternalInput":
            if name != partition_name:
                in_names.append(name)
        elif alloc.kind == "ExternalOutput":
            out_names.append(name)
            shape = tuple(alloc.tensor_shape)
            dtype = mybir.dt.np(alloc.dtype)
            out_avals.append(jax.core.ShapedArray(shape, dtype))
            zero_outs.append(np.zeros(shape, dtype))
    n_params = len(in_names)
    n_outs = len(out_names)
    all_in = list(in_names) + list(out_names)
    if partition_name is not None:
        all_in.append(partition_name)

    def _body(*args):
        operands = list(args)
        if partition_name is not None:
            operands.append(bass2jax.partition_id_tensor())
        return tuple(_bass_exec_p.bind(
            *operands, out_avals=tuple(out_avals), in_names=tuple(all_in),
            out_names=tuple(out_names), lowering_input_output_aliases=(),
            sim_require_finite=True, sim_require_nnan=True, nc=nc))

    devices = jax.devices()[:B]
    mesh = Mesh(np.asarray(devices), ("core",))
    nio = n_params + n_outs
    fn = jax.jit(shard_map(_body, mesh=mesh,
                           in_specs=(PartitionSpec("core"),) * nio,
                           out_specs=(PartitionSpec("core"),) * n_outs,
                           check_rep=False),
                 donate_argnums=tuple(range(n_params, nio)),
                 keep_unused=True)
    concat_in = [np.concatenate([np.asarray(m[in_names[i]]) for m in in_maps],
                                axis=0) for i in range(n_params)]
    shard = NamedSharding(mesh, PartitionSpec("core"))
    dev_in = [jax.device_put(a, shard) for a in concat_in]
    jax.block_until_ready(dev_in)

    def mkouts():
        return [np.zeros((B * z.shape[0], *z.shape[1:]), z.dtype)
                for z in zero_outs]

    def call():
        import time
        a = mkouts()
        t0 = time.perf_counter()
        jax.block_until_ready(fn(*dev_in, *a))
        return time.perf_counter() - t0

    return call


def measure_exec_ns(inputs, rbig=65, n=12):
    """HW time per kernel execution, measured as the slope between a NEFF
    that runs the body once and one that runs it `rbig` times in a For_i
    hardware loop. Inputs are device-resident and outputs donated, so each
    timed call only ships the output buffers; interleaved sampling with
    min-statistics rejects the relay's latency noise."""
    sh, per_core = host_prepare(inputs)
    in_maps = []
    for b in range(B):
        m = dict(sh)
        m.update(per_core[b])
        in_maps.append(m)
    call1 = _timed_fn(build_program(reps=1), in_maps)
    callB = _timed_fn(build_program(reps=rbig), in_maps)
    w1s, wbs = [], []
    fails = 0
    for i in range(n + 2):
        if len(w1s) >= n:
            break
        try:
            if i == 0:
                call1()
                callB()   # warm both executables
                continue
            w1s.append(call1())
            wbs.append(callB())
        except Exception as e:
            fails += 1
            print(f"[timing] round {i} failed: {type(e).__name__}")
            if fails >= 3:
                break
    if not (w1s and wbs):
        return None
    print(f"[timing] w1(ms): {' '.join(f'{x * 1e3:.0f}' for x in w1s)}")
    print(f"[timing] w{rbig}(ms): {' '.join(f'{x * 1e3:.0f}' for x in wbs)}")
    per = (min(wbs) - min(w1s)) / (rbig - 1)
    return max(int(per * 1e9), 0)
